# revision 44
# baseline (speedup 1.0000x reference)
"""Trainium2 Bass kernel for ragged-sequence attention (fp8 stream, flipped MMs).

Per batch b:
    tq     = tanh(query[b] @ W + bias)                      [CA, H]
    scores = key[b] @ tq.T                                  [S, CA]
    alpha  = exp(scores) * (s < seq_len[b])                 [S, CA]
    out[b] = (alpha.T @ value[b]) / alpha.sum(axis=0)[:,None]

Strategy (HBM-bandwidth bound; everything serves DMA bytes):
  - Work items: 128-row sub-chunks of each valid prefix. Each sub yields an
    independent partial numerator [6*128h x 32c] + denominator [32c];
    the host reduces partials per batch and divides.
  - Streams key (pre-scaled x32) and value in fp8 e3m4 for long batches
    (>=3 subs), fp16 for short batches (<=2 subs, where quantization noise
    doesn't average out). tq stays fp16 (mixed-dtype matmul), alpha fp16.
  - All matmuls are oriented so the *moving* (cost-bearing) dim is CA=32:
    scores_T[s,c] += keyT_tile[h,s]^T @ tqT_tile[h,c]   (6 h-tiles)
    num[h,c]      =  value_tile[s,h]^T @ alpha[s,c]     (6 h-tiles)
    den[1,c]      =  ones[s,1]^T       @ alpha[s,c]
  - Ragged masking lives in the data, not the instruction stream: key/value
    rows past seq_len are zero-padded and the ones column (which the
    denominator matmul contracts) is zeroed there, so alpha needs no mask op
    and one Exp activation (scalar bias -1.5, cancels in num/den) serves a
    slot pair -- half the PE<->ACT round-trips.
  - SPMD-uniform module: all cores run identical slot templates
    (c8 fp8 slots + c16 fp16 slots); slot *data* differs per core, dummy
    slots are zero-filled (ones column 0 -> no contribution).
  - key is pre-transposed on the host into [128, 6, 128] h-major tiles;
    value stays s-major [128, 768] with a ones column for the denominator.
"""

import os
import sys

import numpy as np

for _p in ("/opt/trn_rl_repo", "/root/.axon_site/_ro/trn_rl_repo"):
    if os.path.isdir(_p) and _p not in sys.path:
        sys.path.append(_p)

N_CORES = 8
SUB = 128
H = 768
HS = H // 128   # 6
CA = 32
KSCALE = 32.0   # key pre-scale so e3m4 sees normal-range values
SHIFT = -1.5    # exp bias shift (cancels in num/den); keeps fp16 partials safe
F16_SUB_MAX = 2  # batches with <= this many subs stream in fp16

WTQ = HS * CA                  # 192:  tqT
WKV = HS * SUB + H + 1         # 1537: keyT 768 | value 768 | ones 1
W8 = WTQ + WKV                 # 1730: all e3m4
W16 = WTQ + WKV                # 1730: all f16
G8 = 6                         # fp8 slots per chunk (one DMA)
G16 = 1                        # f16 slots per chunk

_module_cache = {}
_last_in_maps = None


def _np_f8():
    import ml_dtypes

    return ml_dtypes.float8_e3m4


def _plan(c8, c16):
    """Slot order: one fp8 first (fast fill), f16 slots early (fat DMAs stay
    off the critical tail), fp8 rest. Every slot is its own input DMA."""
    order = []
    if c8:
        order.append("f8")
    order += ["f16"] * c16
    order += ["f8"] * (c8 - 1) if c8 else []
    return order


def _obgroups(n):
    """Output groups of 2 (keeps out-DMA descriptors >=512B)."""
    gs = [2] * (n // 2)
    if n % 2:
        gs.append(1)
    return gs


def _build_module(c8, c16, depth=2):
    import concourse.mybir as mybir
    import concourse.tile as tile
    from concourse import bacc

    f32 = mybir.dt.float32
    f16 = mybir.dt.float16
    f8 = mybir.dt.float8e3
    AF = mybir.ActivationFunctionType

    order = _plan(c8, c16)
    nslots = c8 + c16

    nc = bacc.Bacc(None, target_bir_lowering=False, enable_asserts=False)
    comb8 = (
        nc.dram_tensor("comb8", [128, c8 * W8], f8, kind="ExternalInput")
        if c8
        else None
    )
    comb16 = (
        nc.dram_tensor("comb16", [128, c16 * W16], f16, kind="ExternalInput")
        if c16
        else None
    )
    # per slot: num [128, 192] f16 + den row-0 [1, 32] f16
    out_d = nc.dram_tensor(
        "outp", [128, nslots * (HS + 1) * CA], f16, kind="ExternalOutput"
    )

    groups = _obgroups(nslots)

    with tile.TileContext(nc) as tc:
        with (
            tc.tile_pool(name="big", bufs=8) as big,
            tc.tile_pool(name="al", bufs=6) as al_pool,
            tc.tile_pool(name="ob", bufs=3) as ob_pool,
            tc.tile_pool(name="ps_s", bufs=4, space="PSUM") as ps_s_pool,
            tc.tile_pool(name="ps_n", bufs=4, space="PSUM") as ps_n_pool,
        ):
            # shared exp-shift bias column (scalar const, one memset)
            shift_t = nc.alloc_sbuf_tensor("shiftc", [128, 1], f32)
            nc.gpsimd.memset(shift_t.ap(), SHIFT)

            # ---- stage the whole slot schedule (tiles + APs) up front ----
            # one input DMA per slot: compute never waits on a fat multi-slot
            # transfer, only on its own 1730B-per-line slice
            slots = []  # per-slot view dicts
            i8 = 0  # fp8-slot ordinal (comb8 offset)
            i16 = 0  # f16-slot ordinal (comb16 offset)
            gidx, m = 0, 0
            ob = None
            for slot, kind in enumerate(order):
                if kind == "f8":
                    ct = big.tile([128, W8], f8, tag="c8", name="ct8")
                    nc.sync.dma_start(
                        out=ct, in_=comb8[:, i8 * W8 : (i8 + 1) * W8]
                    )
                    i8 += 1
                else:
                    ct = big.tile([128, W16], f16, tag="c16", name="ct16")
                    nc.sync.dma_start(
                        out=ct, in_=comb16[:, i16 * W16 : (i16 + 1) * W16]
                    )
                    i16 += 1

                if m == 0:
                    g = groups[gidx]
                    ob = ob_pool.tile(
                        [128, g * (HS + 1) * CA], f16, tag="ob", name="ob"
                    )
                tqb = ct[:, :WTQ]
                base = WTQ
                slots.append(
                    dict(
                        tqv=tqb.rearrange("p (o c) -> p o c", o=HS),
                        ktv=ct[:, base : base + HS * SUB].rearrange(
                            "p (o s) -> p o s", o=HS
                        ),
                        vlv=ct[:, base + HS * SUB : base + HS * SUB + H],
                        ones=ct[:, base + HS * SUB + H : base + WKV],
                        ob=ob,
                        m=m,
                        g=groups[gidx],
                        s0=slot - m,
                        idx=slot,
                    )
                )
                m += 1
                if m == groups[gidx]:
                    gidx += 1
                    m = 0

            # ---- software-pipelined emission over slot PAIRS ----
            # Masking lives in the ones column (host zeroes masked rows) and
            # zero-padded key/value, so exp uses a scalar bias and one
            # activation serves a whole pair -- half the PE<->ACT round-trips.
            pairs = [slots[j : j + 2] for j in range(0, nslots, 2)]

            def emit_scores(pr):
                k = len(pr)
                ps_s = ps_s_pool.tile([128, k * CA], f32, tag="pss", name="pss")
                for m, s in enumerate(pr):
                    for o in range(HS):
                        nc.tensor.matmul(
                            ps_s[:, m * CA : (m + 1) * CA],
                            lhsT=s["ktv"][:, o, :],
                            rhs=s["tqv"][:, o, :],
                            start=(o == 0),
                            stop=(o == HS - 1),
                        )
                al = al_pool.tile([128, k * CA], f16, tag="al", name="al")
                for m, s in enumerate(pr):
                    s["al"] = al[:, m * CA : (m + 1) * CA]
                nc.scalar.activation(
                    out=al,
                    in_=ps_s,
                    func=AF.Exp,
                    bias=shift_t.ap(),
                    scale=1.0 / KSCALE,
                )

            def emit_tail(pr):
                k = len(pr)
                W = (HS + 1) * CA
                # per slot m: num[h-tile o] = value_o^T @ alpha -> [128h, 32c]
                # at cols m*224+o*32; den = ones^T @ alpha -> [1, 32c] at row 0
                # cols m*224+192 (junk in partitions 1-127 there is ignored)
                ps_n = ps_n_pool.tile([128, k * W], f32, tag="psn", name="psn")
                for m, s in enumerate(pr):
                    al = s["al"]
                    for o in range(HS):
                        nc.tensor.matmul(
                            ps_n[:, m * W + o * CA : m * W + (o + 1) * CA],
                            lhsT=s["vlv"][:, o * SUB : (o + 1) * SUB],
                            rhs=al,
                            start=True,
                            stop=True,
                        )
                    nc.tensor.matmul(
                        ps_n[0:1, m * W + HS * CA : (m + 1) * W],
                        lhsT=s["ones"],
                        rhs=al,
                        start=True,
                        stop=True,
                    )
                s = pr[0]
                nc.vector.tensor_copy(out=s["ob"], in_=ps_n)
                # out-DMA on SP: all input descriptor-gens are emitted
                # first, so these waits never delay an input
                s0 = s["s0"]
                nc.sync.dma_start(
                    out=out_d[:, s0 * W : (s0 + len(pr)) * W],
                    in_=s["ob"],
                )

            n = len(pairs)
            for i in range(n + depth):
                if i < n:
                    emit_scores(pairs[i])
                if i - depth >= 0:
                    emit_tail(pairs[i - depth])

    nc.compile()
    return nc


def kernel(key, value, query, seq_len, W, b):
    key = np.ascontiguousarray(np.asarray(key, dtype=np.float32))
    value = np.ascontiguousarray(np.asarray(value, dtype=np.float32))
    query = np.asarray(query, dtype=np.float32)
    W_ = np.asarray(W, dtype=np.float32)
    bias_in = np.asarray(b, dtype=np.float32)
    sl = np.asarray(seq_len).astype(np.int64)

    B, S, H_ = key.shape
    assert H_ == H and S % SUB == 0
    CA_ = query.shape[1]
    assert CA_ == CA

    f8 = _np_f8()

    # host: tiny projection tq[b] = tanh(query[b] @ W + bias) -> [128p, 6o, 32c]
    tq = np.tanh(query.reshape(B * CA, -1) @ W_ + bias_in)
    tq = tq.reshape(B, CA, H).astype(np.float32)
    tqT = {
        bi: np.ascontiguousarray(
            tq[bi].T.reshape(HS, 128, CA).transpose(1, 0, 2)
        ).reshape(128, HS * CA)
        for bi in range(B)
    }

    # work list
    subs8, subs16 = [], []  # (batch, s0, nval)
    for bi in range(B):
        L = int(max(1, min(sl[bi], S)))
        nsub = -(-L // SUB)
        dst = subs16 if nsub <= F16_SUB_MAX else subs8
        for s0 in range(0, L, SUB):
            dst.append((bi, s0, min(SUB, L - s0)))
    n8, n16 = len(subs8), len(subs16)

    # uniform per-core template: prefer moving fp8 leftovers into f16 slots
    # when that shrinks total bytes
    cands = []
    c8a = -(-n8 // N_CORES)
    c16a = -(-n16 // N_CORES)
    cands.append((c8a, c16a))
    c8b = n8 // N_CORES
    c16b = -(-(n16 + (n8 - c8b * N_CORES)) // N_CORES)
    cands.append((c8b, c16b))
    cost = lambda c: c[0] * W8 + c[1] * 2 * W16
    c8, c16 = min(cands, key=cost)
    nslots = c8 + c16

    comb8 = np.zeros((N_CORES, 128, c8 * W8), f8)
    comb16 = np.zeros((N_CORES, 128, c16 * W16), np.float16)
    slot_map = [[] for _ in range(N_CORES)]  # (slot, batch)

    def fill_slot(arr, col0, bi, s0, nval, npdt, kscale):
        # keyT [128p, 6o, 128s]
        kc = np.zeros((SUB, H), np.float32)
        kc[:nval] = key[bi, s0 : s0 + nval] * kscale
        arr[:, col0 : col0 + HS * SUB] = (
            kc.T.reshape(HS, 128, SUB).transpose(1, 0, 2).reshape(128, HS * SUB)
        ).astype(npdt)
        vc = arr[:, col0 + HS * SUB : col0 + HS * SUB + H]
        vc[:nval] = value[bi, s0 : s0 + nval].astype(npdt)
        # ones column doubles as the ragged mask: den = sum(ones * alpha)
        arr[:nval, col0 + WKV - 1] = npdt(1.0)

    def fill_tq(arr, col0, bi, nval, npdt):
        arr[:, col0 : col0 + HS * CA] = tqT[bi].astype(npdt)

    # ordinal -> global slot index (slot order) for output decode
    glob8, glob16 = [], []
    for gi, kind in enumerate(_plan(c8, c16)):
        (glob8 if kind == "f8" else glob16).append(gi)

    # deal fp8 subs: first 8*c8 into fp8 slots, leftovers join the f16 pool
    over8 = subs8[N_CORES * c8 :]
    for idx, (bi, s0, nval) in enumerate(subs8[: N_CORES * c8]):
        c, k = idx // c8, idx % c8
        fill_slot(comb8[c], k * W8 + WTQ, bi, s0, nval, f8, KSCALE)
        fill_tq(comb8[c], k * W8, bi, nval, f8)
        slot_map[c].append((glob8[k], bi))
    for idx, (bi, s0, nval) in enumerate(subs16 + over8):
        c, k = idx // c16, idx % c16
        fill_slot(comb16[c], k * W16 + WTQ, bi, s0, nval, np.float16, KSCALE)
        fill_tq(comb16[c], k * W16, bi, nval, np.float16)
        slot_map[c].append((glob16[k], bi))

    ck = (c8, c16)
    if ck not in _module_cache:
        _module_cache[ck] = _build_module(c8, c16)
    nc = _module_cache[ck]

    from concourse.bass_utils import run_bass_kernel_spmd

    in_maps = []
    for c in range(N_CORES):
        m = {}
        if c8:
            m["comb8"] = comb8[c]
        if c16:
            m["comb16"] = comb16[c]
        in_maps.append(m)
    global _last_in_maps
    _last_in_maps = in_maps
    trace = os.environ.get("BASS_KERNEL_TRACE") == "1"
    kwargs = {}
    if trace:
        kwargs = dict(trace=True, trace_cores=list(range(N_CORES)))
    res = run_bass_kernel_spmd(nc, in_maps, core_ids=list(range(N_CORES)), **kwargs)
    if trace and res.exec_time_ns is not None:
        print(f"HW exec time: {res.exec_time_ns} ns")
        print(f"HW exec time mean: {res.mean_exec_time_ns} ns")

    num = np.zeros((B, CA, H), np.float64)
    den = np.zeros((B, CA), np.float64)
    WS = (HS + 1) * CA
    for c in range(N_CORES):
        parts = res.results[c]["outp"]  # [128, nslots*224] f16
        for k, bi in slot_map[c]:
            blk = parts[:, k * WS : k * WS + HS * CA]
            # [128p, 6o, 32c] -> num[b, c, o*128+p]
            num[bi] += (
                blk.astype(np.float64)
                .reshape(128, HS, CA)
                .transpose(2, 1, 0)
                .reshape(CA, H)
            )
            den[bi] += parts[0, k * WS + HS * CA : (k + 1) * WS].astype(np.float64)
    out = (num / den[:, :, None]).astype(np.float32)
    return out


# revision 49
# speedup vs baseline: 1.0160x; 1.0160x over previous
"""Trainium2 Bass kernel for ragged-sequence attention (fp8 stream, flipped MMs).

Per batch b:
    tq     = tanh(query[b] @ W + bias)                      [CA, H]
    scores = key[b] @ tq.T                                  [S, CA]
    alpha  = exp(scores) * (s < seq_len[b])                 [S, CA]
    out[b] = (alpha.T @ value[b]) / alpha.sum(axis=0)[:,None]

Strategy (HBM-bandwidth bound; everything serves DMA bytes):
  - Work items: 128-row sub-chunks of each valid prefix. Each sub yields an
    independent partial numerator [6*128h x 32c] + denominator [32c];
    the host reduces partials per batch and divides.
  - Streams key (pre-scaled x32) and value in fp8 e3m4 for long batches
    (>=3 subs), fp16 for short batches (<=2 subs, where quantization noise
    doesn't average out). tq stays fp16 (mixed-dtype matmul), alpha fp16.
  - All matmuls are oriented so the *moving* (cost-bearing) dim is CA=32:
    scores_T[s,c] += keyT_tile[h,s]^T @ tqT_tile[h,c]   (6 h-tiles)
    num[h,c]      =  value_tile[s,h]^T @ alpha[s,c]     (6 h-tiles)
    den[1,c]      =  ones[s,1]^T       @ alpha[s,c]
  - Ragged masking lives in the data, not the instruction stream: key/value
    rows past seq_len are zero-padded and the ones column (which the
    denominator matmul contracts) is zeroed there, so alpha needs no mask op
    and one Exp activation (scalar bias -1.5, cancels in num/den) serves a
    slot pair -- half the PE<->ACT round-trips.
  - SPMD-uniform module: all cores run identical slot templates
    (c8 fp8 slots + c16 fp16 slots); slot *data* differs per core, dummy
    slots are zero-filled (ones column 0 -> no contribution).
  - key is pre-transposed on the host into [128, 6, 128] h-major tiles;
    value stays s-major [128, 768] with a ones column for the denominator.
"""

import os
import sys

import numpy as np

for _p in ("/opt/trn_rl_repo", "/root/.axon_site/_ro/trn_rl_repo"):
    if os.path.isdir(_p) and _p not in sys.path:
        sys.path.append(_p)

N_CORES = 8
SUB = 128
H = 768
HS = H // 128   # 6
CA = 32
KSCALE = 32.0   # key pre-scale so e3m4 sees normal-range values
SHIFT = -1.5    # exp bias shift (cancels in num/den); keeps fp16 partials safe
F16_SUB_MAX = 2  # batches with <= this many subs stream in fp16

WTQ = HS * CA                  # 192:  tqT
WKV = HS * SUB + H + 1         # 1537: keyT 768 | value 768 | ones 1
W8 = WTQ + WKV                 # 1730 e3m4 cols: tq | keyT | value | ones
# f16 slot: tq/value/ones in f16, keyT packed as e3m4 bytes in f16 columns
W16 = WTQ + H + 1 + HS * SUB // 2  # 1345 f16 cols
G8 = 6                         # fp8 slots per chunk (one DMA)
G16 = 1                        # f16 slots per chunk

_module_cache = {}
_last_in_maps = None


def _np_f8():
    import ml_dtypes

    return ml_dtypes.float8_e3m4


def _plan(c8, c16):
    """Slot order: one fp8 first (fast fill), f16 slots early (fat DMAs stay
    off the critical tail), fp8 rest. Every slot is its own input DMA."""
    order = []
    if c8:
        order.append("f8")
    order += ["f16"] * c16
    order += ["f8"] * (c8 - 1) if c8 else []
    return order


def _obgroups(n):
    """Output groups of 2 (keeps out-DMA descriptors >=512B)."""
    gs = [2] * (n // 2)
    if n % 2:
        gs.append(1)
    return gs


def _build_module(c8, c16, depth=2):
    import concourse.mybir as mybir
    import concourse.tile as tile
    from concourse import bacc

    f32 = mybir.dt.float32
    f16 = mybir.dt.float16
    f8 = mybir.dt.float8e3
    AF = mybir.ActivationFunctionType

    order = _plan(c8, c16)
    nslots = c8 + c16

    nc = bacc.Bacc(None, target_bir_lowering=False, enable_asserts=False)
    comb8 = (
        nc.dram_tensor("comb8", [128, c8 * W8], f8, kind="ExternalInput")
        if c8
        else None
    )
    comb16 = (
        nc.dram_tensor("comb16", [128, c16 * W16], f16, kind="ExternalInput")
        if c16
        else None
    )
    # per slot: num [128, 192] f16 + den row-0 [1, 32] f16
    out_d = nc.dram_tensor(
        "outp", [128, nslots * (HS + 1) * CA], f16, kind="ExternalOutput"
    )

    groups = _obgroups(nslots)

    with tile.TileContext(nc) as tc:
        with (
            tc.tile_pool(name="big", bufs=8) as big,
            tc.tile_pool(name="al", bufs=6) as al_pool,
            tc.tile_pool(name="ob", bufs=3) as ob_pool,
            tc.tile_pool(name="ps_s", bufs=4, space="PSUM") as ps_s_pool,
            tc.tile_pool(name="ps_n", bufs=4, space="PSUM") as ps_n_pool,
        ):
            # shared exp-shift bias column (scalar const, one memset)
            shift_t = nc.alloc_sbuf_tensor("shiftc", [128, 1], f32)
            nc.gpsimd.memset(shift_t.ap(), SHIFT)

            # ---- stage the whole slot schedule (tiles + APs) up front ----
            # one input DMA per slot: compute never waits on a fat multi-slot
            # transfer, only on its own 1730B-per-line slice
            slots = []  # per-slot view dicts
            i8 = 0  # fp8-slot ordinal (comb8 offset)
            i16 = 0  # f16-slot ordinal (comb16 offset)
            gidx, m = 0, 0
            ob = None
            for slot, kind in enumerate(order):
                if kind == "f8":
                    ct = big.tile([128, W8], f8, tag="c8", name="ct8")
                    nc.sync.dma_start(
                        out=ct, in_=comb8[:, i8 * W8 : (i8 + 1) * W8]
                    )
                    i8 += 1
                else:
                    ct = big.tile([128, W16], f16, tag="c16", name="ct16")
                    nc.sync.dma_start(
                        out=ct, in_=comb16[:, i16 * W16 : (i16 + 1) * W16]
                    )
                    i16 += 1

                if m == 0:
                    g = groups[gidx]
                    ob = ob_pool.tile(
                        [128, g * (HS + 1) * CA], f16, tag="ob", name="ob"
                    )
                if kind == "f8":
                    ktv = ct[:, WTQ : WTQ + HS * SUB]
                    vlv = ct[:, WTQ + HS * SUB : WTQ + HS * SUB + H]
                    ones = ct[:, WTQ + HS * SUB + H : WTQ + WKV]
                else:
                    # key rides as e3m4 bytes bitcast out of f16 columns
                    vlv = ct[:, WTQ : WTQ + H]
                    ones = ct[:, WTQ + H : WTQ + H + 1]
                    ktv = ct[:, WTQ + H + 1 : W16].bitcast(f8)
                slots.append(
                    dict(
                        tqv=ct[:, :WTQ].rearrange("p (o c) -> p o c", o=HS),
                        ktv=ktv.rearrange("p (o s) -> p o s", o=HS),
                        vlv=vlv,
                        ones=ones,
                        ob=ob,
                        m=m,
                        g=groups[gidx],
                        s0=slot - m,
                        idx=slot,
                    )
                )
                m += 1
                if m == groups[gidx]:
                    gidx += 1
                    m = 0

            # ---- software-pipelined emission over slot PAIRS ----
            # Masking lives in the ones column (host zeroes masked rows) and
            # zero-padded key/value, so exp uses a scalar bias and one
            # activation serves a whole pair -- half the PE<->ACT round-trips.
            pairs = [slots[j : j + 2] for j in range(0, nslots, 2)]

            def emit_scores(pr):
                k = len(pr)
                ps_s = ps_s_pool.tile([128, k * CA], f32, tag="pss", name="pss")
                for m, s in enumerate(pr):
                    for o in range(HS):
                        nc.tensor.matmul(
                            ps_s[:, m * CA : (m + 1) * CA],
                            lhsT=s["ktv"][:, o, :],
                            rhs=s["tqv"][:, o, :],
                            start=(o == 0),
                            stop=(o == HS - 1),
                        )
                al = al_pool.tile([128, k * CA], f16, tag="al", name="al")
                for m, s in enumerate(pr):
                    s["al"] = al[:, m * CA : (m + 1) * CA]
                nc.scalar.activation(
                    out=al,
                    in_=ps_s,
                    func=AF.Exp,
                    bias=shift_t.ap(),
                    scale=1.0 / KSCALE,
                )

            def emit_tail(pr):
                k = len(pr)
                W = (HS + 1) * CA
                # per slot m: num[h-tile o] = value_o^T @ alpha -> [128h, 32c]
                # at cols m*224+o*32; den = ones^T @ alpha -> [1, 32c] at row 0
                # cols m*224+192 (junk in partitions 1-127 there is ignored)
                ps_n = ps_n_pool.tile([128, k * W], f32, tag="psn", name="psn")
                for m, s in enumerate(pr):
                    al = s["al"]
                    for o in range(HS):
                        nc.tensor.matmul(
                            ps_n[:, m * W + o * CA : m * W + (o + 1) * CA],
                            lhsT=s["vlv"][:, o * SUB : (o + 1) * SUB],
                            rhs=al,
                            start=True,
                            stop=True,
                        )
                    nc.tensor.matmul(
                        ps_n[0:1, m * W + HS * CA : (m + 1) * W],
                        lhsT=s["ones"],
                        rhs=al,
                        start=True,
                        stop=True,
                    )
                s = pr[0]
                nc.vector.tensor_copy(out=s["ob"], in_=ps_n)
                # out-DMA on SP: all input descriptor-gens are emitted
                # first, so these waits never delay an input
                s0 = s["s0"]
                nc.sync.dma_start(
                    out=out_d[:, s0 * W : (s0 + len(pr)) * W],
                    in_=s["ob"],
                )

            n = len(pairs)
            for i in range(n + depth):
                if i < n:
                    emit_scores(pairs[i])
                if i - depth >= 0:
                    emit_tail(pairs[i - depth])

    nc.compile()
    return nc


def kernel(key, value, query, seq_len, W, b):
    key = np.ascontiguousarray(np.asarray(key, dtype=np.float32))
    value = np.ascontiguousarray(np.asarray(value, dtype=np.float32))
    query = np.asarray(query, dtype=np.float32)
    W_ = np.asarray(W, dtype=np.float32)
    bias_in = np.asarray(b, dtype=np.float32)
    sl = np.asarray(seq_len).astype(np.int64)

    B, S, H_ = key.shape
    assert H_ == H and S % SUB == 0
    CA_ = query.shape[1]
    assert CA_ == CA

    f8 = _np_f8()

    # host: tiny projection tq[b] = tanh(query[b] @ W + bias) -> [128p, 6o, 32c]
    tq = np.tanh(query.reshape(B * CA, -1) @ W_ + bias_in)
    tq = tq.reshape(B, CA, H).astype(np.float32)
    tqT = {
        bi: np.ascontiguousarray(
            tq[bi].T.reshape(HS, 128, CA).transpose(1, 0, 2)
        ).reshape(128, HS * CA)
        for bi in range(B)
    }

    # work list
    subs8, subs16 = [], []  # (batch, s0, nval)
    for bi in range(B):
        L = int(max(1, min(sl[bi], S)))
        nsub = -(-L // SUB)
        dst = subs16 if nsub <= F16_SUB_MAX else subs8
        for s0 in range(0, L, SUB):
            dst.append((bi, s0, min(SUB, L - s0)))
    n8, n16 = len(subs8), len(subs16)

    # uniform per-core template: prefer moving fp8 leftovers into f16 slots
    # when that shrinks total bytes
    cands = []
    c8a = -(-n8 // N_CORES)
    c16a = -(-n16 // N_CORES)
    cands.append((c8a, c16a))
    c8b = n8 // N_CORES
    c16b = -(-(n16 + (n8 - c8b * N_CORES)) // N_CORES)
    cands.append((c8b, c16b))
    cost = lambda c: c[0] * W8 + c[1] * 2 * W16  # bytes/line
    c8, c16 = min(cands, key=cost)
    nslots = c8 + c16

    comb8 = np.zeros((N_CORES, 128, c8 * W8), f8)
    comb16 = np.zeros((N_CORES, 128, c16 * W16), np.float16)
    slot_map = [[] for _ in range(N_CORES)]  # (slot, batch)

    def _keyT8(bi, s0, nval):
        # keyT [128p, 6o*128s] in e3m4, x32 pre-scale
        kc = np.zeros((SUB, H), np.float32)
        kc[:nval] = key[bi, s0 : s0 + nval] * KSCALE
        return np.ascontiguousarray(
            kc.T.reshape(HS, 128, SUB).transpose(1, 0, 2).reshape(128, HS * SUB)
        ).astype(f8)

    def fill_slot8(arr, col0, bi, s0, nval):
        arr[:, col0 : col0 + HS * SUB] = _keyT8(bi, s0, nval)
        vc = arr[:, col0 + HS * SUB : col0 + HS * SUB + H]
        vc[:nval] = value[bi, s0 : s0 + nval].astype(f8)
        # ones column doubles as the ragged mask: den = sum(ones * alpha)
        arr[:nval, col0 + WKV - 1] = f8(1.0)

    def fill_slot16(arr, col0, bi, s0, nval):
        vc = arr[:, col0 : col0 + H]
        vc[:nval] = value[bi, s0 : s0 + nval].astype(np.float16)
        arr[:nval, col0 + H] = np.float16(1.0)
        arr[:, col0 + H + 1 : col0 + H + 1 + HS * SUB // 2] = _keyT8(
            bi, s0, nval
        ).view(np.float16)

    def fill_tq(arr, col0, bi, nval, npdt):
        arr[:, col0 : col0 + HS * CA] = tqT[bi].astype(npdt)

    # ordinal -> global slot index (slot order) for output decode
    glob8, glob16 = [], []
    for gi, kind in enumerate(_plan(c8, c16)):
        (glob8 if kind == "f8" else glob16).append(gi)

    # deal fp8 subs: first 8*c8 into fp8 slots, leftovers join the f16 pool
    over8 = subs8[N_CORES * c8 :]
    for idx, (bi, s0, nval) in enumerate(subs8[: N_CORES * c8]):
        c, k = idx // c8, idx % c8
        fill_slot8(comb8[c], k * W8 + WTQ, bi, s0, nval)
        fill_tq(comb8[c], k * W8, bi, nval, f8)
        slot_map[c].append((glob8[k], bi))
    for idx, (bi, s0, nval) in enumerate(subs16 + over8):
        c, k = idx // c16, idx % c16
        fill_slot16(comb16[c], k * W16 + WTQ, bi, s0, nval)
        fill_tq(comb16[c], k * W16, bi, nval, np.float16)
        slot_map[c].append((glob16[k], bi))

    ck = (c8, c16)
    if ck not in _module_cache:
        _module_cache[ck] = _build_module(c8, c16)
    nc = _module_cache[ck]

    from concourse.bass_utils import run_bass_kernel_spmd

    in_maps = []
    for c in range(N_CORES):
        m = {}
        if c8:
            m["comb8"] = comb8[c]
        if c16:
            m["comb16"] = comb16[c]
        in_maps.append(m)
    global _last_in_maps
    _last_in_maps = in_maps
    trace = os.environ.get("BASS_KERNEL_TRACE") == "1"
    kwargs = {}
    if trace:
        kwargs = dict(trace=True, trace_cores=list(range(N_CORES)))
    res = run_bass_kernel_spmd(nc, in_maps, core_ids=list(range(N_CORES)), **kwargs)
    if trace and res.exec_time_ns is not None:
        print(f"HW exec time: {res.exec_time_ns} ns")
        print(f"HW exec time mean: {res.mean_exec_time_ns} ns")

    num = np.zeros((B, CA, H), np.float64)
    den = np.zeros((B, CA), np.float64)
    WS = (HS + 1) * CA
    for c in range(N_CORES):
        parts = res.results[c]["outp"]  # [128, nslots*224] f16
        for k, bi in slot_map[c]:
            blk = parts[:, k * WS : k * WS + HS * CA]
            # [128p, 6o, 32c] -> num[b, c, o*128+p]
            num[bi] += (
                blk.astype(np.float64)
                .reshape(128, HS, CA)
                .transpose(2, 1, 0)
                .reshape(CA, H)
            )
            den[bi] += parts[0, k * WS + HS * CA : (k + 1) * WS].astype(np.float64)
    out = (num / den[:, :, None]).astype(np.float32)
    return out
